# revision 23
# baseline (speedup 1.0000x reference)
"""Multi-head attention on 8 Trainium2 NeuronCores.

Sharding: core c = (batch n, head-group g); n = c // 4, g = c % 4.
Each core computes attention for its 4 heads of its batch entry plus the
fc_out partial product for those heads' columns of Wo; the host sums the
4 partials per batch (and adds the bias) to unshard.

Schedule: one globally woven PE program driven by a static scheduler.
ScalarE's exp stream (128 tiles of [128,1024], ~1.11us each) is the
pacing resource: score matmul pairs are emitted on a ~1.12us cadence of
cumulative PE stream time for the whole kernel, and everything else
(q/k/v projections, attn@v, fc_out) fills the PE slack between score
emissions.  Blocks run qs-major so the first fc half (columns 0-1023)
can weave into the final attention block's window; the second half runs
in the tail.  xT is DMA'd in L-major 512-chunks so the first scores can
issue ~10us in (vs ~28us when xT streams stripe-major).

PSUM budget (8 banks): phase 1 = psA 2 (projection accumulators) +
psS 4 (double-buffered [128,1024] scores) + psAVe 2 (block-0 attn@v);
after projections finish, psA+psAVe swap for psAV 4 (two blocks in
flight); after the last scores, psS swaps for psC 4 (fc accumulators).
"""

import contextlib as _contextlib
import os
import sys

for _p in ("/opt/trn_rl_repo",):
    if _p not in sys.path and os.path.isdir(_p):
        sys.path.insert(0, _p)

import numpy as np
import ml_dtypes

import concourse.bass as bass
import concourse.mybir as mybir
import concourse.tile as tile
from concourse import bacc
from concourse.bass import ds, ts
from concourse.bass_utils import run_bass_kernel_spmd

BF16 = ml_dtypes.bfloat16
F32 = np.float32

EMBED = 1024
HEADS = 16
HD = 64  # head dim
NB = 2  # batch
L = 2048  # sequence length
NCORES = 8
HPG = 4  # heads per core (group)
NPAIRS = 2  # head pairs per core
ET = EMBED // 128  # 8 contraction tiles for projections
LT = L // 128  # 16 k tiles
QS = 1024  # q superchunk (exp free-dim)
NQS = L // QS  # 2
NLC = L // 512  # 4 512-wide l chunks

SCALE = 1.0 / np.sqrt(np.float32(EMBED))  # 1/32

# scheduler tuning
SC_CAD = 1125.0  # target score-pair cadence in PE stream ns (ScalarE ~1114)
SC_OFF = 1200.0  # stream-time offset of the first score pair
EX_BUFS = 32  # exp tile pool slots
MM512 = 1000.0 / 2.4 * 512  # 213ns: N=512 bf16 matmul stream time
MM256 = 1000.0 / 2.4 * 256

LAST_EXEC_TIME_NS = None
LAST_RESULTS = None

_nc_cache = None


def build_nc():
    """Build + compile the per-core Bass program (same program on all cores)."""
    nc = bacc.Bacc("TRN2")
    f32 = mybir.dt.float32
    bf16 = mybir.dt.bfloat16
    EXP = mybir.ActivationFunctionType.Exp

    xT_d = nc.declare_dram_parameter("xT", [EMBED, L], bf16, isOutput=False)
    wqk_d = nc.declare_dram_parameter("wqk", [4, EMBED, 128], bf16, isOutput=False)
    wv_d = nc.declare_dram_parameter("wv", [EMBED, HPG * HD], bf16, isOutput=False)
    wo_d = nc.declare_dram_parameter("wo", [NPAIRS, ET, 128, 128], bf16, isOutput=False)
    out_d = nc.declare_dram_parameter("out", [EMBED, L], bf16, isOutput=True)
    recip_dram = nc.dram_tensor("recip_dram", [16, 512], bf16)

    with tile.TileContext(nc) as tc:
        with (
            tc.tile_pool(name="expp", bufs=EX_BUFS) as expp,
            tc.tile_pool(name="singles", bufs=1) as singles,
            tc.tile_pool(name="drowp", bufs=3) as drowp,
            tc.tile_pool(name="rbp", bufs=4) as rbp,
            tc.tile_pool(name="shiftp", bufs=3) as shiftp,
            tc.tile_pool(name="outp", bufs=3) as outp,
        ):
            # ---- resident SBUF tensors ----
            xT_sb = singles.tile([128, ET, L], bf16, name="xT_sb")
            wqk_sb = singles.tile([128, 4, ET, 128], bf16, name="wqk_sb")
            wv_sb = singles.tile([128, ET, HPG * HD], bf16, name="wv_sb")
            wo_sb = singles.tile([128, NPAIRS, ET, 128], bf16, name="wo_sb")
            qt_sb = singles.tile([128, NPAIRS, L], bf16, name="qt_sb")
            kt_sb = singles.tile([128, NPAIRS, L], bf16, name="kt_sb")
            v_sb = singles.tile([128, LT, HPG, HD + 1], bf16, name="v_sb")
            outTP_sb = singles.tile([128, NPAIRS, L], bf16, name="outTP_sb")
            num_sb = singles.tile([HD, HPG, L], bf16, name="num_sb")
            warm_sb = singles.tile([1, 8], bf16, name="warm_sb")
            warm2_sb = singles.tile([1, 8], bf16, name="warm2_sb")
            denom_bl = [
                singles.tile([4, 512], f32, name=f"denom{b}") for b in range(4)
            ]
            recip_bl = [
                singles.tile([4, 512], f32, name=f"recip{b}") for b in range(4)
            ]
            recipb_bl = [
                singles.tile([4, 512], bf16, name=f"recipb{b}") for b in range(4)
            ]


            # absorb the exp ACT_TABLE_LOAD + scalar TENSOR_LOAD at t~0
            nc.vector.memset(warm_sb, 0.0)
            nc.scalar.activation(warm2_sb, warm_sb, EXP, scale=1.0)

            # ---- input DMAs: wqk j0/j1, then xT L-major, then the rest ----
            xT_ap = xT_d[:].rearrange("(t p) l -> p t l", p=128)
            wqk_ap = wqk_d[:].rearrange("j (t p) c -> p j t c", p=128)
            for j in range(2):
                nc.sync.dma_start(out=wqk_sb[:, j, :, :], in_=wqk_ap[:, j, :, :])
            # column-half-major, one transfer per half: HWDGE descriptor
            # generation (~1us per dma_start) serializes the queue, so the
            # critical first 2MB (q cols 0-1023 + k tiles 0-7) must be few
            # transfers, not many
            for lh in range(2):
                nc.sync.dma_start(
                    out=xT_sb[:, :, ts(lh, 1024)],
                    in_=xT_ap[:, :, ts(lh, 1024)],
                )
            for j in range(2, 4):
                nc.sync.dma_start(out=wqk_sb[:, j, :, :], in_=wqk_ap[:, j, :, :])
            nc.sync.dma_start(
                out=wv_sb, in_=wv_d[:].rearrange("(t p) c -> p t c", p=128)
            )
            nc.sync.dma_start(
                out=wo_sb, in_=wo_d[:].rearrange("r t p c -> p r t c")
            )

            # ones column for the attn@v denominator rows; v copies write
            # only [..., 0:HD] so this never races with them
            nc.vector.memset(v_sb[:, :, :, HD : HD + 1], 1.0)

            if os.environ.get("KERNEL_GATE_START"):
                # experiment: hold the PE until the xT DMA burst completes by
                # routing wqk j0 through a DVE copy ordered after a read of
                # the last xT chunk (mimics the baseline's quiet start)
                probe = singles.tile([128, 8], bf16, name="probe")
                nc.vector.tensor_copy(probe, xT_sb[:, ET - 1, L - 8 : L])
                wqk_j0_stage = singles.tile([128, ET, 128], bf16, name="wqkj0s")
                nc.vector.tensor_copy(wqk_j0_stage, wqk_sb[:, 0, :, :])
                nc.vector.tensor_copy(wqk_sb[:, 0, :, :], wqk_j0_stage)

            # ---- PSUM pools (LIFO stack: psS outermost, then per-phase) ----
            _psS_stack = _contextlib.ExitStack()
            psS = _psS_stack.enter_context(
                tc.tile_pool(name="psS", bufs=2, space="PSUM")
            )
            _psA_stack = _contextlib.ExitStack()
            psA = _psA_stack.enter_context(
                tc.tile_pool(name="psA", bufs=2, space="PSUM")
            )
            _psAVe_stack = _contextlib.ExitStack()
            psAVe = _psAVe_stack.enter_context(
                tc.tile_pool(name="psAVe", bufs=2, space="PSUM")
            )
            psAV = None
            psAV2 = None
            psF = None
            _psAV_stack = _contextlib.ExitStack()
            _psF_stack = _contextlib.ExitStack()

            # ---- block plan: qs-major so fc cols 0-1023 unblock early ----
            blocks = [
                (qs, pair, side)
                for qs in range(NQS)
                for pair in range(NPAIRS)
                for side in range(2)
            ]
            NSC = len(blocks) * LT  # 128 score/exp tiles

            def sc_key(i):
                qs, pair, side = blocks[i // LT]
                return (pair, qs, side, i % LT)

            # ---- emission helpers ----
            ex_store = {}
            av_tiles = {}

            def emit_sc_exp(i):
                pair, qs, side, k = sc_key(i)
                base = side * HD
                sc = psS.tile([128, QS], f32, tag="sc", name="sc")
                for half in range(2):
                    nc.tensor.matmul(
                        sc[:, ts(half, 512)],
                        kt_sb[base : base + HD, pair, ts(k, 128)],
                        qt_sb[base : base + HD, pair, ds(qs * QS + half * 512, 512)],
                        start=True,
                        stop=True,
                    )
                ex = expp.tile([128, QS], bf16, tag="exp", name="ex")
                nc.scalar.activation(ex, sc, EXP, scale=float(SCALE))
                ex_store[i] = ex

            ridx = 0
            rbase = {}

            def emit_av(c):
                nonlocal ridx
                b = c // LT
                k = c % LT
                qs, pair, side = blocks[b]
                h_local = pair * 2 + side
                blk = pair * NQS + qs
                if k == 0:
                    pool = psAVe if b == 0 else (psAV if b < 7 else psAV2)
                    av_tiles[b] = [
                        pool.tile([128, 512], f32, tag="av", name=f"av{b}_{h}")
                        for h in range(2)
                    ]
                    if side == 0:
                        rbase[blk] = ridx
                av = av_tiles[b]
                ex = ex_store.pop(c)
                for half in range(2):
                    nc.tensor.matmul(
                        av[half][0 : HD + 1, :],
                        v_sb[:, k, h_local, :],
                        ex[:, ts(half, 512)],
                        start=(k == 0),
                        stop=(k == LT - 1),
                    )
                if k == LT - 1:
                    # evacuate numerators + denominator rows
                    for half in range(2):
                        avt = av[half]
                        col0 = qs * QS + half * 512
                        nc.vector.tensor_copy(
                            num_sb[:, h_local, ds(col0, 512)], avt[0:HD, :]
                        )
                        dr = drowp.tile([HD + 1, 512], f32, tag="dr", name="dr")
                        nc.vector.tensor_copy(
                            dr[HD : HD + 1, :], avt[HD : HD + 1, :]
                        )
                        r = ridx - rbase[blk]
                        nc.sync.dma_start(
                            out=denom_bl[blk][r : r + 1, :],
                            in_=dr[HD : HD + 1, :],
                        )
                        ridx += 1
                    del av_tiles[b]
                    if side == 1:
                        emit_norm(pair, qs, blk)

            def emit_norm(pair, qs, blk):
                # reciprocal of the 4 denominator rows, bf16, DRAM-bounce
                # partition-broadcast, multiply into outTP
                r0 = rbase[blk]
                nc.vector.reciprocal_approx_fast(recip_bl[blk], denom_bl[blk])
                nc.vector.tensor_copy(recipb_bl[blk], recip_bl[blk])
                nc.sync.dma_start(
                    out=recip_dram[r0 : r0 + 4, :], in_=recipb_bl[blk]
                )
                j = r0
                for side2 in range(2):
                    h2 = pair * 2 + side2
                    for half in range(2):
                        col0 = qs * QS + half * 512
                        rb = rbp.tile([HD, 512], bf16, tag="rb", name="rb")
                        nc.sync.dma_start(
                            out=rb,
                            in_=recip_dram[j : j + 1, :].to_broadcast([HD, 512]),
                        )
                        if side2 == 0:
                            nc.vector.tensor_mul(
                                outTP_sb[0:HD, pair, ds(col0, 512)],
                                num_sb[0:HD, h2, ds(col0, 512)],
                                rb,
                            )
                        else:
                            # odd head: normalize into a temp, then
                            # DMA-shift to partitions 64-127
                            tmp = shiftp.tile([HD, 512], bf16, tag="sh", name="sh")
                            nc.vector.tensor_mul(
                                tmp,
                                num_sb[0:HD, h2, ds(col0, 512)],
                                rb,
                            )
                            nc.sync.dma_start(
                                out=outTP_sb[HD:128, pair, ds(col0, 512)],
                                in_=tmp,
                            )
                        j += 1

            fc_ev = 0
            fc_ob = {}

            def emit_fc_half(lcp, et, half, pool=None):
                # [128,512] fc accumulator; the two halves of a (lcp, et)
                # chunk share one [128,1024] ob staging tile and one out DMA
                nonlocal fc_ev
                pool = pool or psF
                tag = "av" if pool is psAV2 else "fc"
                fps = pool.tile([128, 512], f32, tag=tag, name=f"fc{et}_{lcp}")
                col0 = lcp * 1024 + half * 512
                for pair in range(NPAIRS):
                    nc.tensor.matmul(
                        fps,
                        wo_sb[:, pair, et, :],
                        outTP_sb[:, pair, ds(col0, 512)],
                        start=(pair == 0),
                        stop=(pair == NPAIRS - 1),
                    )
                if half == 0:
                    fc_ob[(lcp, et)] = outp.tile(
                        [128, 1024], bf16, tag="ob", name="ob"
                    )
                ob = fc_ob[(lcp, et)]
                if fc_ev % 2 == 0:
                    nc.scalar.copy(ob[:, ts(half, 512)], fps)
                else:
                    nc.vector.tensor_copy(ob[:, ts(half, 512)], fps)
                fc_ev += 1
                if half == 1:
                    nc.sync.dma_start(
                        out=out_d[ts(et, 128), ds(lcp * 1024, 1024)],
                        in_=fc_ob.pop((lcp, et)),
                    )

            # ---- filler units (single matmuls for fine-grain weaving) ----
            # order: j0 lc0,lc1 | j1 lc0-3 | j2 lc0,lc1 | j3 lc0-3 |
            #        j0 lc2,lc3 | j2 lc2,lc3 | v lt0-15
            j_done = set()
            v_done_lt = -1
            j_ps = {}

            def mk_j_unit(j, lc, et):
                def emit():
                    if et == 0:
                        j_ps[(j, lc)] = psA.tile(
                            [128, 512], f32, tag="pa", name=f"j{j}lc{lc}"
                        )
                    nc.tensor.matmul(
                        j_ps[(j, lc)],
                        wqk_sb[:, j, et, :],
                        xT_sb[:, et, ts(lc, 512)],
                        start=(et == 0),
                        stop=(et == ET - 1),
                    )
                    if et == ET - 1:
                        dst = qt_sb if j % 2 == 0 else kt_sb
                        nc.vector.tensor_copy(
                            dst[:, j // 2, ts(lc, 512)], j_ps.pop((j, lc))
                        )
                        j_done.add((j, lc))

                return emit, MM512

            def mk_v_unit(lt, et):
                def emit():
                    nonlocal v_done_lt
                    if et == 0:
                        j_ps[("v", lt)] = psA.tile(
                            [128, 512], f32, tag="pa", name=f"v{lt}"
                        )
                    pv = j_ps[("v", lt)][:, : HPG * HD]
                    nc.tensor.matmul(
                        pv,
                        xT_sb[:, et, ts(lt, 128)],
                        wv_sb[:, et, :],
                        start=(et == 0),
                        stop=(et == ET - 1),
                    )
                    if et == ET - 1:
                        nc.vector.tensor_copy(
                            v_sb[:, lt, :, 0:HD],
                            pv.rearrange("p (h d) -> p h d", h=HPG),
                        )
                        j_ps.pop(("v", lt))
                        v_done_lt = lt

                return emit, MM256

            fillers = []
            j_order = [
                (0, 0), (0, 1), (1, 0), (1, 1),  # first-half columns
                (1, 2), (1, 3),                  # k second half (sc k>=8)
                (2, 0), (2, 1), (3, 0), (3, 1),  # pair 1 first half
                (3, 2), (3, 3),
                (0, 2), (0, 3), (2, 2), (2, 3),  # q second half (qs=1)
            ]
            for j, lc in j_order:
                fillers += [mk_j_unit(j, lc, et) for et in range(ET)]
            for lt in range(LT):
                fillers += [mk_v_unit(lt, et) for et in range(ET)]
            fillers = fillers[::-1]  # pop() from the end

            def sc_ready(i):
                pair, qs, side, k = sc_key(i)
                return (
                    (2 * pair, 2 * qs) in j_done
                    and (2 * pair, 2 * qs + 1) in j_done
                    and (2 * pair + 1, k // 4) in j_done
                )

            def av_ready(c, sc_i, swapped):
                if c >= NSC or c > sc_i - 2:
                    return False
                if c % LT > v_done_lt:
                    return False
                return (c // LT == 0) or swapped

            # ---- the weave ----
            pe_time = 0.0
            sc_i = 0
            consumed = 0
            swapped = False
            fc0 = [(0, et, h) for et in range(ET) for h in range(2)][::-1]
            fc1 = [(1, et, h) for et in range(ET) for h in range(2)][::-1]
            av_since_fc = 0
            spins = 0

            while sc_i < NSC or consumed < NSC or fc0:
                spins += 1
                assert spins < 20000, (
                    f"scheduler wedged: sc_i={sc_i} consumed={consumed} "
                    f"fillers={len(fillers)} fc0={len(fc0)} swapped={swapped}"
                )
                if not swapped and not fillers and consumed >= LT:
                    # projections + block 0 done: swap psA/psAVe -> psAV
                    _psAVe_stack.close()
                    _psA_stack.close()
                    psAV = _psAV_stack.enter_context(
                        tc.tile_pool(name="psAV", bufs=4, space="PSUM")
                    )
                    swapped = True
                if psF is None and consumed == 7 * LT:
                    # blocks 1-6 done: shrink attn@v to 2 banks for block 7,
                    # freeing 2 banks for the fc accumulators
                    _psAV_stack.close()
                    psAV2 = _psF_stack.enter_context(
                        tc.tile_pool(name="psAV2", bufs=2, space="PSUM")
                    )
                    psF = _psF_stack.enter_context(
                        tc.tile_pool(name="psF", bufs=2, space="PSUM")
                    )
                # 1) forced attn@v to keep the exp pool from wrapping
                if consumed < sc_i - (EX_BUFS - 2) and av_ready(
                    consumed, sc_i, swapped
                ):
                    emit_av(consumed)
                    consumed += 1
                    av_since_fc += 1
                    pe_time += 2 * MM512
                    continue
                # 2) scores on cadence
                if (
                    sc_i < NSC
                    and pe_time >= SC_OFF + sc_i * SC_CAD
                    and sc_ready(sc_i)
                    and consumed >= sc_i - (EX_BUFS - 2)
                ):
                    emit_sc_exp(sc_i)
                    sc_i += 1
                    pe_time += 2 * MM512
                    continue
                # 3) projection / v fillers
                if fillers:
                    emit, cost = fillers.pop()
                    emit()
                    pe_time += cost
                    continue
                # 4) fc cols 0-1023 woven between block-7 attn@v steps
                if (
                    psF is not None
                    and fc0
                    and (av_since_fc >= 1 or not av_ready(consumed, sc_i, swapped))
                ):
                    lcp, et, h = fc0.pop()
                    emit_fc_half(lcp, et, h)
                    av_since_fc = 0
                    pe_time += 2 * MM512
                    continue
                # 5) attn@v opportunistically
                if consumed < NSC and av_ready(consumed, sc_i, swapped):
                    emit_av(consumed)
                    consumed += 1
                    av_since_fc += 1
                    pe_time += 2 * MM512
                    continue
                # nothing ready: advance virtual time to the next cadence slot
                if sc_i < NSC:
                    pe_time = max(pe_time, SC_OFF + sc_i * SC_CAD)
                else:
                    break

            # drain any leftovers (defensive; loop above should cover all)
            while consumed < NSC:
                emit_av(consumed)
                consumed += 1
            while fc0:
                lcp, et, h = fc0.pop()
                emit_fc_half(lcp, et, h)

            # ---- tail: fc cols 1024-2047 (block 7's attn@v banks are free
            # now, so alternate pools to halve accumulator WAR stalls) ----
            fci = 0
            while fc1:
                lcp, et, h = fc1.pop()
                emit_fc_half(lcp, et, h, pool=(psF if fci % 2 == 0 else psAV2))
                fci += 1

            _psF_stack.close()
            _psS_stack.close()

    nc.compile()
    return nc


def get_nc():
    global _nc_cache
    if _nc_cache is None:
        _nc_cache = build_nc()
    return _nc_cache


def make_core_inputs(x, Wq, Wk, Wv, Wo, bo):
    """Build the 8 per-core input maps from the full-size inputs."""
    x = np.asarray(x, F32)
    Wq = np.asarray(Wq, F32)
    Wk = np.asarray(Wk, F32)
    Wv = np.asarray(Wv, F32)
    Wo = np.asarray(Wo, F32)
    bo = np.asarray(bo, F32)

    xT_b = [np.ascontiguousarray(x[n].T).astype(BF16) for n in range(NB)]

    in_maps = []
    for c in range(NCORES):
        n, g = divmod(c, HPG)
        heads = [g * HPG + i for i in range(HPG)]

        wqk = np.empty((4, EMBED, 128), F32)
        for j in range(4):
            pair, qk = divmod(j, 2)
            hA = heads[2 * pair]
            hB = heads[2 * pair + 1]
            W = Wq if qk == 0 else Wk
            wqk[j, :, 0:HD] = W[hA * HD : (hA + 1) * HD, :].T
            wqk[j, :, HD:128] = W[hB * HD : (hB + 1) * HD, :].T

        wv = np.concatenate(
            [Wv[h * HD : (h + 1) * HD, :].T for h in heads], axis=1
        )  # [1024, 256]

        wo = np.empty((NPAIRS, ET, 128, 128), F32)
        for pair in range(NPAIRS):
            hA = heads[2 * pair]
            hB = heads[2 * pair + 1]
            for et in range(ET):
                blk = Wo[et * 128 : (et + 1) * 128, :]
                wo[pair, et, 0:HD, :] = blk[:, hA * HD : (hA + 1) * HD].T
                wo[pair, et, HD:128, :] = blk[:, hB * HD : (hB + 1) * HD].T

        in_maps.append(
            {
                "xT": xT_b[n],
                "wqk": wqk.astype(BF16),
                "wv": wv.astype(BF16),
                "wo": wo.astype(BF16),
            }
        )
    return in_maps


def combine_outputs(results, bo):
    """Sum the per-core fc_out partials, add bias, transpose to [N, L, E]."""
    out = np.empty((NB, L, EMBED), F32)
    for n in range(NB):
        acc = results[n * HPG]["out"].astype(F32)
        for g in range(1, HPG):
            acc = acc + results[n * HPG + g]["out"].astype(F32)
        out[n] = acc.T + np.asarray(bo, F32)
    return out


def kernel(x, Wq, Wk, Wv, Wo, bo):
    global LAST_EXEC_TIME_NS, LAST_RESULTS
    nc = get_nc()
    in_maps = make_core_inputs(x, Wq, Wk, Wv, Wo, bo)
    trace = bool(os.environ.get("KERNEL_TRACE"))
    kw = {}
    if trace:
        kw["trace"] = True
        kw["trace_cores"] = list(range(NCORES))
    res = run_bass_kernel_spmd(nc, in_maps, list(range(NCORES)), **kw)
    LAST_EXEC_TIME_NS = res.exec_time_ns
    LAST_RESULTS = res
    return combine_outputs(res.results, bo)


# revision 29
# speedup vs baseline: 1.0484x; 1.0484x over previous
"""Multi-head attention on 8 Trainium2 NeuronCores.

Sharding: core c = (batch n, head-group g); n = c // 4, g = c % 4.
Each core computes attention for its 4 heads of its batch entry plus the
fc_out partial product for those heads' columns of Wo; the host sums the
4 partials per batch (and adds the bias) to unshard.

Schedule: one globally woven PE program driven by a static scheduler.
ScalarE's exp stream (128 tiles of [128,1024], ~1.11us each) is the
pacing resource: score matmul pairs are emitted on a ~1.12us cadence of
cumulative PE stream time for the whole kernel, and everything else
(q/k/v projections, attn@v, fc_out) fills the PE slack between score
emissions.  Blocks run qs-major so the first fc half (columns 0-1023)
can weave into the final attention block's window; the second half runs
in the tail.  xT is DMA'd in L-major 512-chunks so the first scores can
issue ~10us in (vs ~28us when xT streams stripe-major).

PSUM budget (8 banks): phase 1 = psA 2 (projection accumulators) +
psS 4 (double-buffered [128,1024] scores) + psAVe 2 (block-0 attn@v);
after projections finish, psA+psAVe swap for psAV 4 (two blocks in
flight); after the last scores, psS swaps for psC 4 (fc accumulators).
"""

import contextlib as _contextlib
import os
import sys

for _p in ("/opt/trn_rl_repo",):
    if _p not in sys.path and os.path.isdir(_p):
        sys.path.insert(0, _p)

import numpy as np
import ml_dtypes

import concourse.bass as bass
import concourse.mybir as mybir
import concourse.tile as tile
from concourse import bacc
from concourse.bass import ds, ts
from concourse.bass_utils import run_bass_kernel_spmd

BF16 = ml_dtypes.bfloat16
F32 = np.float32

EMBED = 1024
HEADS = 16
HD = 64  # head dim
NB = 2  # batch
L = 2048  # sequence length
NCORES = 8
HPG = 4  # heads per core (group)
NPAIRS = 2  # head pairs per core
ET = EMBED // 128  # 8 contraction tiles for projections
LT = L // 128  # 16 k tiles
QS = 1024  # q superchunk (exp free-dim)
NQS = L // QS  # 2
NLC = L // 512  # 4 512-wide l chunks

SCALE = 1.0 / np.sqrt(np.float32(EMBED))  # 1/32

# scheduler tuning
SC_CAD = 1125.0  # target score-pair cadence in PE stream ns (ScalarE ~1114)
SC_OFF = 1200.0  # stream-time offset of the first score pair
EX_BUFS = 29  # exp tile pool slots
MM512 = 1000.0 / 2.4 * 512  # 213ns: N=512 bf16 matmul stream time
MM256 = 1000.0 / 2.4 * 256

LAST_EXEC_TIME_NS = None
LAST_RESULTS = None

_nc_cache = None


def build_nc():
    """Build + compile the per-core Bass program (same program on all cores)."""
    nc = bacc.Bacc("TRN2")
    f32 = mybir.dt.float32
    bf16 = mybir.dt.bfloat16
    EXP = mybir.ActivationFunctionType.Exp

    xT_d = nc.declare_dram_parameter("xT", [EMBED, L], bf16, isOutput=False)
    wqk_d = nc.declare_dram_parameter("wqk", [4, EMBED, 128], bf16, isOutput=False)
    wv_d = nc.declare_dram_parameter("wv", [EMBED, HPG * HD], bf16, isOutput=False)
    wo_d = nc.declare_dram_parameter("wo", [NPAIRS, ET, 128, 128], bf16, isOutput=False)
    out_d = nc.declare_dram_parameter("out", [EMBED, L], bf16, isOutput=True)
    recip_dram = nc.dram_tensor("recip_dram", [16, 512], bf16)

    with tile.TileContext(nc) as tc:
        with (
            tc.tile_pool(name="expp", bufs=EX_BUFS) as expp,
            tc.tile_pool(name="singles", bufs=1) as singles,
            tc.tile_pool(name="drowp", bufs=3) as drowp,
            tc.tile_pool(name="rbp", bufs=4) as rbp,
            tc.tile_pool(name="shiftp", bufs=3) as shiftp,
            tc.tile_pool(name="outp", bufs=2) as outp,
        ):
            # ---- resident SBUF tensors ----
            xT_sb = singles.tile([128, ET, L], bf16, name="xT_sb")
            wqk_sb = singles.tile([128, 4, ET, 128], bf16, name="wqk_sb")
            wv_sb = singles.tile([128, ET, HPG * HD], bf16, name="wv_sb")
            wo_sb = singles.tile([128, NPAIRS, ET, 128], bf16, name="wo_sb")
            qt_sb = singles.tile([128, NPAIRS, L], bf16, name="qt_sb")
            kt_sb = singles.tile([128, NPAIRS, L], bf16, name="kt_sb")
            v_sb = singles.tile([128, LT, HPG, HD + 1], bf16, name="v_sb")
            outTP_sb = singles.tile([128, NPAIRS, L], bf16, name="outTP_sb")
            num_sb = singles.tile([HD, HPG, L], bf16, name="num_sb")
            obig_sb = singles.tile([128, ET, 1024], bf16, name="obig_sb")
            warm_sb = singles.tile([1, 8], bf16, name="warm_sb")
            warm2_sb = singles.tile([1, 8], bf16, name="warm2_sb")
            denom_bl = [
                singles.tile([4, 512], f32, name=f"denom{b}") for b in range(4)
            ]
            recip_bl = [
                singles.tile([4, 512], f32, name=f"recip{b}") for b in range(4)
            ]
            recipb_bl = [
                singles.tile([4, 512], bf16, name=f"recipb{b}") for b in range(4)
            ]


            # absorb the exp ACT_TABLE_LOAD + scalar TENSOR_LOAD at t~0
            nc.vector.memset(warm_sb, 0.0)
            nc.scalar.activation(warm2_sb, warm_sb, EXP, scale=1.0)

            # ---- input DMAs: wqk j0/j1, then xT L-major, then the rest ----
            xT_ap = xT_d[:].rearrange("(t p) l -> p t l", p=128)
            wqk_ap = wqk_d[:].rearrange("j (t p) c -> p j t c", p=128)
            for j in range(2):
                nc.sync.dma_start(out=wqk_sb[:, j, :, :], in_=wqk_ap[:, j, :, :])
            # column-half-major, one transfer per half: HWDGE descriptor
            # generation (~1us per dma_start) serializes the queue, so the
            # critical first 2MB (q cols 0-1023 + k tiles 0-7) must be few
            # transfers, not many
            for lh in range(2):
                for eh in range(2):
                    nc.sync.dma_start(
                        out=xT_sb[:, ts(eh, 4), ts(lh, 1024)],
                        in_=xT_ap[:, ts(eh, 4), ts(lh, 1024)],
                    )
            for j in range(2, 4):
                nc.sync.dma_start(out=wqk_sb[:, j, :, :], in_=wqk_ap[:, j, :, :])
            nc.sync.dma_start(
                out=wv_sb, in_=wv_d[:].rearrange("(t p) c -> p t c", p=128)
            )
            nc.sync.dma_start(
                out=wo_sb, in_=wo_d[:].rearrange("r t p c -> p r t c")
            )

            # ones column for the attn@v denominator rows; v copies write
            # only [..., 0:HD] so this never races with them
            nc.vector.memset(v_sb[:, :, :, HD : HD + 1], 1.0)

            if os.environ.get("KERNEL_GATE_START"):
                # experiment: hold the PE until the xT DMA burst completes by
                # routing wqk j0 through a DVE copy ordered after a read of
                # the last xT chunk (mimics the baseline's quiet start)
                probe = singles.tile([128, 8], bf16, name="probe")
                nc.vector.tensor_copy(probe, xT_sb[:, ET - 1, L - 8 : L])
                wqk_j0_stage = singles.tile([128, ET, 128], bf16, name="wqkj0s")
                nc.vector.tensor_copy(wqk_j0_stage, wqk_sb[:, 0, :, :])
                nc.vector.tensor_copy(wqk_sb[:, 0, :, :], wqk_j0_stage)

            # ---- PSUM pools (LIFO stack: psS outermost, then per-phase) ----
            _psS_stack = _contextlib.ExitStack()
            psS = _psS_stack.enter_context(
                tc.tile_pool(name="psS", bufs=2, space="PSUM")
            )
            _psA_stack = _contextlib.ExitStack()
            psA = _psA_stack.enter_context(
                tc.tile_pool(name="psA", bufs=2, space="PSUM")
            )
            _psAVe_stack = _contextlib.ExitStack()
            psAVe = _psAVe_stack.enter_context(
                tc.tile_pool(name="psAVe", bufs=2, space="PSUM")
            )
            psAV = None
            psAV2 = None
            psF = None
            _psAV_stack = _contextlib.ExitStack()
            _psF_stack = _contextlib.ExitStack()

            # ---- block plan: qs-major so fc cols 0-1023 unblock early ----
            blocks = [
                (qs, pair, side)
                for qs in range(NQS)
                for pair in range(NPAIRS)
                for side in range(2)
            ]
            NSC = len(blocks) * LT  # 128 score/exp tiles

            def sc_key(i):
                qs, pair, side = blocks[i // LT]
                return (pair, qs, side, i % LT)

            # ---- emission helpers ----
            ex_store = {}
            av_tiles = {}

            def emit_sc_exp(i):
                pair, qs, side, k = sc_key(i)
                base = side * HD
                sc = psS.tile([128, QS], f32, tag="sc", name="sc")
                for half in range(2):
                    nc.tensor.matmul(
                        sc[:, ts(half, 512)],
                        kt_sb[base : base + HD, pair, ts(k, 128)],
                        qt_sb[base : base + HD, pair, ds(qs * QS + half * 512, 512)],
                        start=True,
                        stop=True,
                    )
                ex = expp.tile([128, QS], bf16, tag="exp", name="ex")
                nc.scalar.activation(ex, sc, EXP, scale=float(SCALE))
                ex_store[i] = ex

            ridx = 0
            rbase = {}

            def emit_av(c):
                nonlocal ridx
                b = c // LT
                k = c % LT
                qs, pair, side = blocks[b]
                h_local = pair * 2 + side
                blk = pair * NQS + qs
                if k == 0:
                    pool = psAVe if b == 0 else (psAV if b < 7 else psAV2)
                    av_tiles[b] = [
                        pool.tile([128, 512], f32, tag="av", name=f"av{b}_{h}")
                        for h in range(2)
                    ]
                    if side == 0:
                        rbase[blk] = ridx
                av = av_tiles[b]
                ex = ex_store.pop(c)
                for half in range(2):
                    nc.tensor.matmul(
                        av[half][0 : HD + 1, :],
                        v_sb[:, k, h_local, :],
                        ex[:, ts(half, 512)],
                        start=(k == 0),
                        stop=(k == LT - 1),
                    )
                if k == LT - 1:
                    # evacuate numerators + denominator rows
                    for half in range(2):
                        avt = av[half]
                        col0 = qs * QS + half * 512
                        nc.vector.tensor_copy(
                            num_sb[:, h_local, ds(col0, 512)], avt[0:HD, :]
                        )
                        dr = drowp.tile([HD + 1, 512], f32, tag="dr", name="dr")
                        nc.vector.tensor_copy(
                            dr[HD : HD + 1, :], avt[HD : HD + 1, :]
                        )
                        r = ridx - rbase[blk]
                        nc.sync.dma_start(
                            out=denom_bl[blk][r : r + 1, :],
                            in_=dr[HD : HD + 1, :],
                        )
                        ridx += 1
                    del av_tiles[b]
                    if side == 1:
                        emit_norm(pair, qs, blk)

            def emit_norm(pair, qs, blk):
                # reciprocal of the 4 denominator rows, bf16, DRAM-bounce
                # partition-broadcast, multiply into outTP
                r0 = rbase[blk]
                nc.vector.reciprocal_approx_fast(recip_bl[blk], denom_bl[blk])
                nc.vector.tensor_copy(recipb_bl[blk], recip_bl[blk])
                nc.sync.dma_start(
                    out=recip_dram[r0 : r0 + 4, :], in_=recipb_bl[blk]
                )
                j = r0
                for side2 in range(2):
                    h2 = pair * 2 + side2
                    for half in range(2):
                        col0 = qs * QS + half * 512
                        rb = rbp.tile([HD, 512], bf16, tag="rb", name="rb")
                        nc.sync.dma_start(
                            out=rb,
                            in_=recip_dram[j : j + 1, :].to_broadcast([HD, 512]),
                        )
                        if side2 == 0:
                            nc.vector.tensor_mul(
                                outTP_sb[0:HD, pair, ds(col0, 512)],
                                num_sb[0:HD, h2, ds(col0, 512)],
                                rb,
                            )
                        else:
                            # odd head: normalize into a temp, then
                            # DMA-shift to partitions 64-127
                            tmp = shiftp.tile([HD, 512], bf16, tag="sh", name="sh")
                            nc.vector.tensor_mul(
                                tmp,
                                num_sb[0:HD, h2, ds(col0, 512)],
                                rb,
                            )
                            nc.sync.dma_start(
                                out=outTP_sb[HD:128, pair, ds(col0, 512)],
                                in_=tmp,
                            )
                        j += 1

            fc_ev = 0
            fc_done = [0, 0]
            fc_ob = {}
            out_ap = out_d[:].rearrange("(t p) l -> p t l", p=128)

            def emit_fc_half(lcp, et, half, pool=None):
                # [128,512] fc accumulator; evacuations land in a staging
                # tile and each column half ships as ONE 2MB DMA (HWDGE
                # descriptor generation serializes the queue, so many small
                # out-DMAs would gate the tail by ~1us each)
                nonlocal fc_ev
                pool = pool or psF
                tag = "av" if pool is psAV2 else "fc"
                fps = pool.tile([128, 512], f32, tag=tag, name=f"fc{et}_{lcp}")
                col0 = lcp * 1024 + half * 512
                for pair in range(NPAIRS):
                    nc.tensor.matmul(
                        fps,
                        wo_sb[:, pair, et, :],
                        outTP_sb[:, pair, ds(col0, 512)],
                        start=(pair == 0),
                        stop=(pair == NPAIRS - 1),
                    )
                if lcp == 0:
                    # fc0 overlaps block 7's attn@v: per-chunk DMAs are fine
                    if half == 0:
                        fc_ob[et] = outp.tile(
                            [128, 1024], bf16, tag="ob", name="ob"
                        )
                    dst = fc_ob[et][:, ts(half, 512)]
                else:
                    # fc1 is the tail: stage and ship as ONE 2MB DMA
                    dst = obig_sb[:, et, ts(half, 512)]
                if fc_ev % 2 == 0:
                    nc.scalar.copy(dst, fps)
                else:
                    nc.vector.tensor_copy(dst, fps)
                fc_ev += 1
                fc_done[lcp] += 1
                if lcp == 0 and half == 1:
                    nc.sync.dma_start(
                        out=out_d[ts(et, 128), ds(0, 1024)],
                        in_=fc_ob.pop(et),
                    )
                if lcp == 1 and fc_done[1] == 2 * ET:
                    nc.sync.dma_start(
                        out=out_ap[:, :, ts(1, 1024)],
                        in_=obig_sb,
                    )

            # ---- filler units (single matmuls for fine-grain weaving) ----
            # order: j0 lc0,lc1 | j1 lc0-3 | j2 lc0,lc1 | j3 lc0-3 |
            #        j0 lc2,lc3 | j2 lc2,lc3 | v lt0-15
            j_done = set()
            v_done_lt = -1
            j_ps = {}

            def mk_j_unit(j, lc, et):
                def emit():
                    if et == 0:
                        j_ps[(j, lc)] = psA.tile(
                            [128, 512], f32, tag="pa", name=f"j{j}lc{lc}"
                        )
                    nc.tensor.matmul(
                        j_ps[(j, lc)],
                        wqk_sb[:, j, et, :],
                        xT_sb[:, et, ts(lc, 512)],
                        start=(et == 0),
                        stop=(et == ET - 1),
                    )
                    if et == ET - 1:
                        dst = qt_sb if j % 2 == 0 else kt_sb
                        nc.vector.tensor_copy(
                            dst[:, j // 2, ts(lc, 512)], j_ps.pop((j, lc))
                        )
                        j_done.add((j, lc))

                return emit, MM512

            def mk_v_unit(lt, et):
                def emit():
                    nonlocal v_done_lt
                    if et == 0:
                        j_ps[("v", lt)] = psA.tile(
                            [128, 512], f32, tag="pa", name=f"v{lt}"
                        )
                    pv = j_ps[("v", lt)][:, : HPG * HD]
                    nc.tensor.matmul(
                        pv,
                        xT_sb[:, et, ts(lt, 128)],
                        wv_sb[:, et, :],
                        start=(et == 0),
                        stop=(et == ET - 1),
                    )
                    if et == ET - 1:
                        nc.vector.tensor_copy(
                            v_sb[:, lt, :, 0:HD],
                            pv.rearrange("p (h d) -> p h d", h=HPG),
                        )
                        j_ps.pop(("v", lt))
                        v_done_lt = lt

                return emit, MM256

            fillers = []
            j_order = [
                (0, 0), (0, 1), (1, 0), (1, 1),  # first-half columns
                (1, 2), (1, 3),                  # k second half (sc k>=8)
                (2, 0), (2, 1), (3, 0), (3, 1),  # pair 1 first half
                (3, 2), (3, 3),
                (0, 2), (0, 3), (2, 2), (2, 3),  # q second half (qs=1)
            ]
            for j, lc in j_order:
                fillers += [mk_j_unit(j, lc, et) for et in range(ET)]
            for lt in range(LT):
                fillers += [mk_v_unit(lt, et) for et in range(ET)]
            fillers = fillers[::-1]  # pop() from the end

            def sc_ready(i):
                pair, qs, side, k = sc_key(i)
                return (
                    (2 * pair, 2 * qs) in j_done
                    and (2 * pair, 2 * qs + 1) in j_done
                    and (2 * pair + 1, k // 4) in j_done
                )

            def av_ready(c, sc_i, swapped):
                if c >= NSC or c > sc_i - 2:
                    return False
                if c % LT > v_done_lt:
                    return False
                return (c // LT == 0) or swapped

            # ---- the weave ----
            pe_time = 0.0
            sc_i = 0
            consumed = 0
            swapped = False
            fc0 = [(0, et, h) for et in range(ET) for h in range(2)][::-1]
            fc1 = [(1, et, h) for et in range(ET) for h in range(2)][::-1]
            av_since_fc = 0
            spins = 0

            while sc_i < NSC or consumed < NSC or fc0:
                spins += 1
                assert spins < 20000, (
                    f"scheduler wedged: sc_i={sc_i} consumed={consumed} "
                    f"fillers={len(fillers)} fc0={len(fc0)} swapped={swapped}"
                )
                if not swapped and not fillers and consumed >= LT:
                    # projections + block 0 done: swap psA/psAVe -> psAV
                    _psAVe_stack.close()
                    _psA_stack.close()
                    psAV = _psAV_stack.enter_context(
                        tc.tile_pool(name="psAV", bufs=4, space="PSUM")
                    )
                    swapped = True
                if psF is None and consumed == 7 * LT:
                    # blocks 1-6 done: shrink attn@v to 2 banks for block 7,
                    # freeing 2 banks for the fc accumulators
                    _psAV_stack.close()
                    psAV2 = _psF_stack.enter_context(
                        tc.tile_pool(name="psAV2", bufs=2, space="PSUM")
                    )
                    psF = _psF_stack.enter_context(
                        tc.tile_pool(name="psF", bufs=2, space="PSUM")
                    )
                # 1) forced attn@v to keep the exp pool from wrapping
                if consumed < sc_i - (EX_BUFS - 2) and av_ready(
                    consumed, sc_i, swapped
                ):
                    emit_av(consumed)
                    consumed += 1
                    av_since_fc += 1
                    pe_time += 2 * MM512
                    continue
                # 2) scores on cadence
                if (
                    sc_i < NSC
                    and pe_time >= SC_OFF + sc_i * SC_CAD
                    and sc_ready(sc_i)
                    and consumed >= sc_i - (EX_BUFS - 2)
                ):
                    emit_sc_exp(sc_i)
                    sc_i += 1
                    pe_time += 2 * MM512
                    continue
                # 3) projection / v fillers
                if fillers:
                    emit, cost = fillers.pop()
                    emit()
                    pe_time += cost
                    continue
                # 4) fc cols 0-1023 woven between block-7 attn@v steps
                if (
                    psF is not None
                    and fc0
                    and (av_since_fc >= 1 or not av_ready(consumed, sc_i, swapped))
                ):
                    lcp, et, h = fc0.pop()
                    emit_fc_half(lcp, et, h)
                    av_since_fc = 0
                    pe_time += 2 * MM512
                    continue
                # 5) attn@v opportunistically
                if consumed < NSC and av_ready(consumed, sc_i, swapped):
                    emit_av(consumed)
                    consumed += 1
                    av_since_fc += 1
                    pe_time += 2 * MM512
                    continue
                # nothing ready: advance virtual time to the next cadence slot
                if sc_i < NSC:
                    pe_time = max(pe_time, SC_OFF + sc_i * SC_CAD)
                else:
                    break

            # drain any leftovers (defensive; loop above should cover all)
            while consumed < NSC:
                emit_av(consumed)
                consumed += 1
            while fc0:
                lcp, et, h = fc0.pop()
                emit_fc_half(lcp, et, h)

            # ---- tail: fc cols 1024-2047 (block 7's attn@v banks are free
            # now, so alternate pools to halve accumulator WAR stalls) ----
            fci = 0
            while fc1:
                lcp, et, h = fc1.pop()
                emit_fc_half(lcp, et, h, pool=(psF if fci % 2 == 0 else psAV2))
                fci += 1

            _psF_stack.close()
            _psS_stack.close()

    nc.compile()
    return nc


def get_nc():
    global _nc_cache
    if _nc_cache is None:
        _nc_cache = build_nc()
    return _nc_cache


def make_core_inputs(x, Wq, Wk, Wv, Wo, bo):
    """Build the 8 per-core input maps from the full-size inputs."""
    x = np.asarray(x, F32)
    Wq = np.asarray(Wq, F32)
    Wk = np.asarray(Wk, F32)
    Wv = np.asarray(Wv, F32)
    Wo = np.asarray(Wo, F32)
    bo = np.asarray(bo, F32)

    xT_b = [np.ascontiguousarray(x[n].T).astype(BF16) for n in range(NB)]

    in_maps = []
    for c in range(NCORES):
        n, g = divmod(c, HPG)
        heads = [g * HPG + i for i in range(HPG)]

        wqk = np.empty((4, EMBED, 128), F32)
        for j in range(4):
            pair, qk = divmod(j, 2)
            hA = heads[2 * pair]
            hB = heads[2 * pair + 1]
            W = Wq if qk == 0 else Wk
            wqk[j, :, 0:HD] = W[hA * HD : (hA + 1) * HD, :].T
            wqk[j, :, HD:128] = W[hB * HD : (hB + 1) * HD, :].T

        wv = np.concatenate(
            [Wv[h * HD : (h + 1) * HD, :].T for h in heads], axis=1
        )  # [1024, 256]

        wo = np.empty((NPAIRS, ET, 128, 128), F32)
        for pair in range(NPAIRS):
            hA = heads[2 * pair]
            hB = heads[2 * pair + 1]
            for et in range(ET):
                blk = Wo[et * 128 : (et + 1) * 128, :]
                wo[pair, et, 0:HD, :] = blk[:, hA * HD : (hA + 1) * HD].T
                wo[pair, et, HD:128, :] = blk[:, hB * HD : (hB + 1) * HD].T

        in_maps.append(
            {
                "xT": xT_b[n],
                "wqk": wqk.astype(BF16),
                "wv": wv.astype(BF16),
                "wo": wo.astype(BF16),
            }
        )
    return in_maps


def combine_outputs(results, bo):
    """Sum the per-core fc_out partials, add bias, transpose to [N, L, E]."""
    out = np.empty((NB, L, EMBED), F32)
    for n in range(NB):
        acc = results[n * HPG]["out"].astype(F32)
        for g in range(1, HPG):
            acc = acc + results[n * HPG + g]["out"].astype(F32)
        out[n] = acc.T + np.asarray(bo, F32)
    return out


def kernel(x, Wq, Wk, Wv, Wo, bo):
    global LAST_EXEC_TIME_NS, LAST_RESULTS
    nc = get_nc()
    in_maps = make_core_inputs(x, Wq, Wk, Wv, Wo, bo)
    trace = bool(os.environ.get("KERNEL_TRACE"))
    kw = {}
    if trace:
        kw["trace"] = True
        kw["trace_cores"] = list(range(NCORES))
    res = run_bass_kernel_spmd(nc, in_maps, list(range(NCORES)), **kw)
    LAST_EXEC_TIME_NS = res.exec_time_ns
    LAST_RESULTS = res
    return combine_outputs(res.results, bo)


# revision 35
# speedup vs baseline: 1.0558x; 1.0071x over previous
"""Multi-head attention on 8 Trainium2 NeuronCores.

Sharding: core c = (batch n, head-group g); n = c // 4, g = c % 4.
Each core computes attention for its 4 heads of its batch entry plus the
fc_out partial product for those heads' columns of Wo; the host sums the
4 partials per batch (and adds the bias) to unshard.

Per-core pipeline (all matmuls bf16, accumulation f32 in PSUM):
  A) qT/kT projections head-pair-stacked ([d,L] layout, pair in partition
     halves 0-63 / 64-127), v projection in [k, d] layout with a ones
     column appended per head (accumulates the softmax denominator for
     free during attn@v). Scores+exp for the first (pair, q-superchunk)
     are woven into the projection loops so ScalarE (the exp bottleneck,
     ~143us of work) starts as early as possible.
  B) scoresT in [k, q] layout (K=64 row-tiled pairs: both heads of a pair
     run concurrently in the PE array), exp on ScalarE straight out of
     PSUM (scale=1/32; no max subtraction needed: scores ~ N(0, 1/16)),
     attn@v accumulated over k tiles into [d+1, q] PSUM (row 64 =
     denominator). Per-(pair,qs) normalization is inlined: reciprocal of
     the 4 denominator rows, DRAM-bounce partition-broadcast, multiply.
     Odd heads are DMA-shifted to partitions 64-127 to form K=128 pairs
     for fc.
  C) fc_out partial = WoPair.T @ outTP with K=128 head pairs; PSUM
     evacuations alternate ScalarE/VectorE; bias is applied on the host
     while summing the partials.
"""

import contextlib as _contextlib
import os
import sys

for _p in ("/opt/trn_rl_repo",):
    if _p not in sys.path and os.path.isdir(_p):
        sys.path.insert(0, _p)

import numpy as np
import ml_dtypes

import concourse.bass as bass
import concourse.mybir as mybir
import concourse.tile as tile
from concourse import bacc
from concourse.bass import ds, ts
from concourse.bass_utils import run_bass_kernel_spmd

BF16 = ml_dtypes.bfloat16
F32 = np.float32

EMBED = 1024
HEADS = 16
HD = 64  # head dim
NB = 2  # batch
L = 2048  # sequence length
NCORES = 8
HPG = 4  # heads per core (group)
NPAIRS = 2  # head pairs per core
ET = EMBED // 128  # 8 contraction tiles for projections
LT = L // 128  # 16 k tiles
QS = 1024  # q superchunk (exp free-dim)
NQS = L // QS  # 2
NLC = L // 512  # 4 512-wide l chunks

N_EARLY = 32  # early-emitted scores/exp steps; MUST be <= expp bufs

SCALE = 1.0 / np.sqrt(np.float32(EMBED))  # 1/32

LAST_EXEC_TIME_NS = None
LAST_RESULTS = None

_nc_cache = None


def build_nc():
    """Build + compile the per-core Bass program (same program on all cores)."""
    nc = bacc.Bacc("TRN2")
    f32 = mybir.dt.float32
    bf16 = mybir.dt.bfloat16
    EXP = mybir.ActivationFunctionType.Exp

    xT_d = nc.declare_dram_parameter("xT", [EMBED, L], bf16, isOutput=False)
    # weights arrive partition-major from the host so every DMA row is a
    # contiguous 2-8KB run (256B rows serialize the DGE and gate startup)
    wqk_d = nc.declare_dram_parameter("wqk", [128, 4, ET, 128], bf16, isOutput=False)
    wv_d = nc.declare_dram_parameter("wv", [128, ET, HPG * HD], bf16, isOutput=False)
    wo_d = nc.declare_dram_parameter("wo", [128, NPAIRS, ET, 128], bf16, isOutput=False)
    out_d = nc.declare_dram_parameter("out", [EMBED, L], bf16, isOutput=True)
    recip_dram = nc.dram_tensor("recip_dram", [16, 512], bf16)

    with tile.TileContext(nc) as tc:
        with (
            tc.tile_pool(name="expp", bufs=N_EARLY) as expp,
            tc.tile_pool(name="singles", bufs=1) as singles,
            tc.tile_pool(name="drowp", bufs=3) as drowp,
            tc.tile_pool(name="rbp", bufs=4) as rbp,
            tc.tile_pool(name="shiftp", bufs=3) as shiftp,
            tc.tile_pool(name="outp", bufs=3) as outp,
        ):
            # ---- resident SBUF tensors ----
            xT_sb = singles.tile([128, ET, L], bf16, name="xT_sb")
            wqk_sb = singles.tile([128, 4, ET, 128], bf16, name="wqk_sb")
            wv_sb = singles.tile([128, ET, HPG * HD], bf16, name="wv_sb")
            wo_sb = singles.tile([128, NPAIRS, ET, 128], bf16, name="wo_sb")
            qt_sb = singles.tile([128, NPAIRS, L], bf16, name="qt_sb")
            kt_sb = singles.tile([128, NPAIRS, L], bf16, name="kt_sb")
            v_sb = singles.tile([128, LT, HPG, HD + 1], bf16, name="v_sb")
            outTP_sb = singles.tile([128, NPAIRS, L], bf16, name="outTP_sb")
            num_sb = singles.tile([HD, HPG, L], bf16, name="num_sb")
            # per-(pair,qs) denominator blocks: 4 rows each, base partition 0
            denom_bl = [
                singles.tile([4, 512], f32, name=f"denom{b}") for b in range(4)
            ]
            recip_bl = [
                singles.tile([4, 512], f32, name=f"recip{b}") for b in range(4)
            ]
            recipb_bl = [
                singles.tile([4, 512], bf16, name=f"recipb{b}") for b in range(4)
            ]

            # ---- input DMAs, ordered so compute starts early ----
            xT_ap = xT_d[:].rearrange("(t p) l -> p t l", p=128)
            for j in range(2):
                nc.sync.dma_start(out=wqk_sb[:, j, :, :], in_=wqk_d[:, j, :, :])
            for et in range(ET):
                nc.sync.dma_start(out=xT_sb[:, et, :], in_=xT_ap[:, et, :])
            nc.sync.dma_start(out=wv_sb, in_=wv_d[:])
            for j in range(2, 4):
                nc.sync.dma_start(out=wqk_sb[:, j, :, :], in_=wqk_d[:, j, :, :])
            nc.sync.dma_start(out=wo_sb, in_=wo_d[:])

            # scores PSUM pool spans phases A+B only; closed before fc so
            # its banks are free for psC
            _psS_stack = _contextlib.ExitStack()
            psS = _psS_stack.enter_context(
                tc.tile_pool(name="psS", bufs=2, space="PSUM")
            )

            _psAV_stack = _contextlib.ExitStack()
            _psAV3_stack = _contextlib.ExitStack()
            psAV3 = None

            ex_store = {}  # (pair, qs, side, k) -> exp tile emitted early
            sc_emitted = set()
            av_tiles = {}
            av_done = set()

            def get_av(pair, qs, side):
                key = (pair, qs, side)
                pool = psAV if key == (0, 0, 0) else psAV3
                if key not in av_tiles:
                    av_tiles[key] = [
                        pool.tile(
                            [128, 512],
                            f32,
                            tag="av",
                            name=f"av{pair}{qs}{side}{h}",
                        )
                        for h in range(2)
                    ]
                return av_tiles[key]

            _weave_iter = iter(range(LT))

            def emit_av_weave():
                # per j2/j3 step: two attn@v k-tiles for (pair0,qs0,side0)
                # plus the matching look-ahead scores/exp for (pair0,qs1,
                # side0) - pops and pushes balance so the exp pool stays
                # exactly full and ScalarE never drains
                for _ in range(2):
                    k = next(_weave_iter, None)
                    if k is None:
                        return
                    av = get_av(0, 0, 0)
                    ex = ex_store.pop((0, 0, 0, k))
                    for half in range(2):
                        nc.tensor.matmul(
                            av[half][0 : HD + 1, :],
                            v_sb[:, k, 0, :],
                            ex[:, ts(half, 512)],
                            start=(k == 0),
                            stop=(k == LT - 1),
                        )
                    lkey = (0, 1, 0, k)
                    sc_emitted.add(lkey)
                    ex_store[lkey] = emit_sc_exp(*lkey)
                    if k == LT - 1:
                        av_done.add((0, 0, 0))

            def emit_sc_exp(pair, qs, side, k):
                base = side * HD
                sc = psS.tile([128, QS], f32, tag="sc", name=f"sc{side}")
                for half in range(2):
                    nc.tensor.matmul(
                        sc[:, ts(half, 512)],
                        kt_sb[base : base + HD, pair, ts(k, 128)],
                        qt_sb[base : base + HD, pair, ds(qs * QS + half * 512, 512)],
                        start=True,
                        stop=True,
                    )
                ex = expp.tile([128, QS], bf16, tag="exp", name="ex")
                nc.scalar.activation(ex, sc, EXP, scale=float(SCALE))
                return ex

            # early-emit list: scores+exp for (pair0, qs0) woven into the
            # v-projection and j2/j3 loops so ScalarE starts early.
            # Capped at the exp pool size: an early exp whose slot reuse
            # depends on a phase-B attn@v consumer would deadlock the PE
            # FIFO (attn@v sits behind phase-A matmuls).
            early = [(0, 0, s, k) for k in range(LT) for s in range(2)]
            early = early[:N_EARLY]

            def emit_sc_exp_pair(pair, qs, k):
                # both sides of a head pair, matmuls interleaved so the
                # (0,0) and (64,0) row-group tiles overlap in the PE array
                scs = [
                    psS.tile([128, QS], f32, tag="sc", name=f"sc{s}")
                    for s in range(2)
                ]
                for half in range(2):
                    for side in range(2):
                        base = side * HD
                        nc.tensor.matmul(
                            scs[side][:, ts(half, 512)],
                            kt_sb[base : base + HD, pair, ts(k, 128)],
                            qt_sb[
                                base : base + HD,
                                pair,
                                ds(qs * QS + half * 512, 512),
                            ],
                            start=True,
                            stop=True,
                        )
                out = []
                for side in range(2):
                    ex = expp.tile([128, QS], bf16, tag="exp", name="ex")
                    nc.scalar.activation(ex, scs[side], EXP, scale=float(SCALE))
                    out.append(ex)
                return out

            def emit_early():
                if len(early) >= 2 and early[0][:3] == (0, 0, 0):
                    k = early[0][3]
                    if early[1] == (0, 0, 1, k):
                        k0, k1 = early.pop(0), early.pop(0)
                        exs = emit_sc_exp_pair(0, 0, k)
                        sc_emitted.add(k0)
                        sc_emitted.add(k1)
                        ex_store[k0], ex_store[k1] = exs[0], exs[1]
                        return
                if early:
                    key = early.pop(0)
                    sc_emitted.add(key)
                    ex_store[key] = emit_sc_exp(*key)

            # ================= Phase A: projections =================
            # j0/j1 keep the 4-bank lc-inner order (paced by the xT DMA
            # stream); v and j2/j3 run single-bank so 4 PSUM banks stay
            # free and phase-B attn@v accumulators can start during A.
            with tc.tile_pool(name="psA4", bufs=4, space="PSUM") as psA4:
                for j in range(2):
                    pst = [
                        psA4.tile([128, 512], f32, tag="ps", name=f"qk{j}_{lc}")
                        for lc in range(NLC)
                    ]
                    for et in range(ET):
                        for lc in range(NLC):
                            nc.tensor.matmul(
                                pst[lc],
                                wqk_sb[:, j, et, :],
                                xT_sb[:, et, ts(lc, 512)],
                                start=(et == 0),
                                stop=(et == ET - 1),
                            )
                    dst = qt_sb if j == 0 else kt_sb
                    for lc in range(NLC):
                        nc.vector.tensor_copy(dst[:, 0, ts(lc, 512)], pst[lc])

            psAV = _psAV_stack.enter_context(
                tc.tile_pool(name="psAV", bufs=2, space="PSUM")
            )
            with tc.tile_pool(name="psA2", bufs=2, space="PSUM") as psA2:
                # v in [k, d] layout, 4 heads side by side
                for lt in range(LT):
                    emit_early()
                    pv = psA2.tile([128, 512], f32, tag="ps2", name=f"v{lt}")
                    pv = pv[:, : HPG * HD]
                    for et in range(ET):
                        nc.tensor.matmul(
                            pv,
                            xT_sb[:, et, ts(lt, 128)],
                            wv_sb[:, et, :],
                            start=(et == 0),
                            stop=(et == ET - 1),
                        )
                    nc.vector.tensor_copy(
                        v_sb[:, lt, :, 0:HD],
                        pv.rearrange("p (h d) -> p h d", h=HPG),
                    )
                    emit_early()
                nc.vector.memset(v_sb[:, :, :, HD : HD + 1], 1.0)
                for j in range(2, 4):
                    dst = qt_sb if j == 2 else kt_sb
                    for lc in range(NLC):
                        pst = psA2.tile(
                            [128, 512], f32, tag="ps2", name=f"qk{j}_{lc}"
                        )
                        for et in range(ET):
                            nc.tensor.matmul(
                                pst,
                                wqk_sb[:, j, et, :],
                                xT_sb[:, et, ts(lc, 512)],
                                start=(et == 0),
                                stop=(et == ET - 1),
                            )
                        nc.vector.tensor_copy(dst[:, 1, ts(lc, 512)], pst)
                        emit_av_weave()

            # ================= Phase B: attention =================
            # Uniform pipeline over 8 side-blocks (pair, qs, side). During
            # side-block i's attn@v k-loop we emit scores/exp for
            # side-block i+2, so ScalarE (the 143us exp bottleneck) keeps
            # streaming and the 32-slot exp pool stays exactly full.
            side_blocks = [
                (p, q, s) for p in range(NPAIRS) for q in range(NQS)
                for s in range(2)
            ]
            ridx = 0
            rbase = {}
            for i, (pair, qs, side) in enumerate(side_blocks):
                blk = pair * NQS + qs
                if side == 0:
                    rbase[blk] = ridx
                h_local = pair * 2 + side
                av = get_av(pair, qs, side)
                if (pair, qs, side) not in av_done:
                    for k in range(LT):
                        if i + 2 < len(side_blocks):
                            lkey = side_blocks[i + 2] + (k,)
                            if lkey not in sc_emitted:
                                sc_emitted.add(lkey)
                                ex_store[lkey] = emit_sc_exp(*lkey)
                        ex = ex_store.pop((pair, qs, side, k))
                        for half in range(2):
                            nc.tensor.matmul(
                                av[half][0 : HD + 1, :],
                                v_sb[:, k, h_local, :],
                                ex[:, ts(half, 512)],
                                start=(k == 0),
                                stop=(k == LT - 1),
                            )
                # evacuate this head's numerators + denominator rows
                for half in range(2):
                    avt = av[half]
                    col0 = qs * QS + half * 512
                    nc.vector.tensor_copy(
                        num_sb[:, h_local, ds(col0, 512)], avt[0:HD, :]
                    )
                    dr = drowp.tile([HD + 1, 512], f32, tag="dr", name="dr")
                    nc.vector.tensor_copy(
                        dr[HD : HD + 1, :], avt[HD : HD + 1, :]
                    )
                    nc.sync.dma_start(
                        out=denom_bl[blk][ridx - rbase[blk] : ridx - rbase[blk] + 1, :],
                        in_=dr[HD : HD + 1, :],
                    )
                    ridx += 1
                if (pair, qs, side) == (0, 0, 0) and psAV3 is None:
                    _psAV_stack.close()
                    psAV3 = _psAV3_stack.enter_context(
                        tc.tile_pool(name="psAV3", bufs=3, space="PSUM")
                    )
                if side != 1:
                    continue
                # normalize this (pair, qs): reciprocal of the 4 rows,
                # bf16, DRAM-bounce partition-broadcast, multiply
                r0 = rbase[blk]
                nc.vector.reciprocal_approx_fast(recip_bl[blk], denom_bl[blk])
                nc.vector.tensor_copy(recipb_bl[blk], recip_bl[blk])
                nc.sync.dma_start(out=recip_dram[r0:ridx, :], in_=recipb_bl[blk])
                j = r0
                for side2 in range(2):
                    h2 = pair * 2 + side2
                    for half in range(2):
                        col0 = qs * QS + half * 512
                        rb = rbp.tile([HD, 512], bf16, tag="rb", name="rb")
                        nc.sync.dma_start(
                            out=rb,
                            in_=recip_dram[j : j + 1, :].to_broadcast([HD, 512]),
                        )
                        if side2 == 0:
                            nc.vector.tensor_mul(
                                outTP_sb[0:HD, pair, ds(col0, 512)],
                                num_sb[0:HD, h2, ds(col0, 512)],
                                rb,
                            )
                        else:
                            # odd head: normalize into a temp, then
                            # DMA-shift to partitions 64-127
                            tmp = shiftp.tile(
                                [HD, 512], bf16, tag="sh", name="sh"
                            )
                            nc.vector.tensor_mul(
                                tmp,
                                num_sb[0:HD, h2, ds(col0, 512)],
                                rb,
                            )
                            nc.sync.dma_start(
                                out=outTP_sb[HD:128, pair, ds(col0, 512)],
                                in_=tmp,
                            )
                        j += 1
            _psAV3_stack.close()

            # warm-keeper: dense dummy matmuls carry the PE through the
            # final normalize window so fc starts at full clock (HAM
            # re-throttles after ~3.4us of PE idle)
            warm = psS.tile([128, 512], f32, tag="sc", name="warm")
            for _ in range(24):
                nc.tensor.matmul(
                    warm,
                    wo_sb[:, 0, 0, :],
                    outTP_sb[:, 0, 0:512],
                    start=True,
                    stop=True,
                )

            _psS_stack.close()  # free scores banks before fc

            # ================= Phase C: fc_out partial =================
            # bias is applied on the host during unsharding; evacuations
            # alternate ScalarE/VectorE in 1024-wide chunks to shorten the
            # drain chain after the last matmul
            out_ap = out_d[:].rearrange("(t p) l -> p t l", p=128)
            with tc.tile_pool(name="psC", bufs=4, space="PSUM") as psC:
                for lcp in range(2):
                    for et in range(ET):
                        fps = psC.tile(
                            [128, 1024], f32, tag="fc", name=f"fc{et}_{lcp}"
                        )
                        for half in range(2):
                            for pair in range(NPAIRS):
                                nc.tensor.matmul(
                                    fps[:, ts(half, 512)],
                                    wo_sb[:, pair, et, :],
                                    outTP_sb[
                                        :, pair, ds(lcp * 1024 + half * 512, 512)
                                    ],
                                    start=(pair == 0),
                                    stop=(pair == NPAIRS - 1),
                                )
                        if lcp == 0:
                            ob = outp.tile(
                                [128, 1024], bf16, tag="ob", name="ob"
                            )
                        else:
                            # the tail half stages into xT_sb (dead after
                            # phase A) and ships as a single 2MB DMA:
                            # per-chunk DMAs cost ~1us of serialized
                            # descriptor generation each
                            ob = xT_sb[:, et, 0:1024]
                        if et % 2 == 0:
                            nc.scalar.copy(ob, fps)
                        else:
                            nc.vector.tensor_copy(ob, fps)
                        if lcp == 0:
                            nc.sync.dma_start(
                                out=out_d[ts(et, 128), ds(0, 1024)], in_=ob
                            )
                        elif et == ET - 1:
                            nc.sync.dma_start(
                                out=out_ap[:, :, ts(1, 1024)],
                                in_=xT_sb[:, :, 0:1024],
                            )

    nc.compile()
    return nc


def get_nc():
    global _nc_cache
    if _nc_cache is None:
        _nc_cache = build_nc()
    return _nc_cache


def make_core_inputs(x, Wq, Wk, Wv, Wo, bo):
    """Build the 8 per-core input maps from the full-size inputs."""
    x = np.asarray(x, F32)
    Wq = np.asarray(Wq, F32)
    Wk = np.asarray(Wk, F32)
    Wv = np.asarray(Wv, F32)
    Wo = np.asarray(Wo, F32)
    bo = np.asarray(bo, F32)

    xT_b = [np.ascontiguousarray(x[n].T).astype(BF16) for n in range(NB)]

    in_maps = []
    for c in range(NCORES):
        n, g = divmod(c, HPG)
        heads = [g * HPG + i for i in range(HPG)]

        wqk = np.empty((4, EMBED, 128), F32)
        for j in range(4):
            pair, qk = divmod(j, 2)
            hA = heads[2 * pair]
            hB = heads[2 * pair + 1]
            W = Wq if qk == 0 else Wk
            wqk[j, :, 0:HD] = W[hA * HD : (hA + 1) * HD, :].T
            wqk[j, :, HD:128] = W[hB * HD : (hB + 1) * HD, :].T

        wv = np.concatenate(
            [Wv[h * HD : (h + 1) * HD, :].T for h in heads], axis=1
        )  # [1024, 256]

        wo = np.empty((NPAIRS, ET, 128, 128), F32)
        for pair in range(NPAIRS):
            hA = heads[2 * pair]
            hB = heads[2 * pair + 1]
            for et in range(ET):
                blk = Wo[et * 128 : (et + 1) * 128, :]
                wo[pair, et, 0:HD, :] = blk[:, hA * HD : (hA + 1) * HD].T
                wo[pair, et, HD:128, :] = blk[:, hB * HD : (hB + 1) * HD].T

        # partition-major relayouts: [p, ...] with contiguous per-p rows
        wqk_t = np.ascontiguousarray(
            wqk.reshape(4, ET, 128, 128).transpose(2, 0, 1, 3)
        )
        wv_t = np.ascontiguousarray(
            wv.reshape(ET, 128, HPG * HD).transpose(1, 0, 2)
        )
        wo_t = np.ascontiguousarray(wo.transpose(2, 0, 1, 3))

        in_maps.append(
            {
                "xT": xT_b[n],
                "wqk": wqk_t.astype(BF16),
                "wv": wv_t.astype(BF16),
                "wo": wo_t.astype(BF16),
            }
        )
    return in_maps


def combine_outputs(results, bo):
    """Sum the per-core fc_out partials, add bias, transpose to [N, L, E]."""
    out = np.empty((NB, L, EMBED), F32)
    for n in range(NB):
        acc = results[n * HPG]["out"].astype(F32)
        for g in range(1, HPG):
            acc = acc + results[n * HPG + g]["out"].astype(F32)
        out[n] = acc.T + np.asarray(bo, F32)
    return out


def kernel(x, Wq, Wk, Wv, Wo, bo):
    global LAST_EXEC_TIME_NS, LAST_RESULTS
    nc = get_nc()
    in_maps = make_core_inputs(x, Wq, Wk, Wv, Wo, bo)
    trace = bool(os.environ.get("KERNEL_TRACE"))
    kw = {}
    if trace:
        kw["trace"] = True
        kw["trace_cores"] = list(range(NCORES))
    res = run_bass_kernel_spmd(nc, in_maps, list(range(NCORES)), **kw)
    LAST_EXEC_TIME_NS = res.exec_time_ns
    LAST_RESULTS = res
    return combine_outputs(res.results, bo)

